# revision 16
# baseline (speedup 1.0000x reference)
"""CRF loss (forward-algorithm partition + gold-path score) on 8 trn2 NeuronCores.

Strategy
--------
Denominator (log-partition, ~99.6% of reference FLOPs): the logsumexp scan is a
matmul in exp space:  alpha_t = log( exp(trans).T @ exp(alpha_{t-1}) ) + e_t.
Keeping the state in exp space, each step is one PE matmul with constant
weights W = exp(trans - C) plus one elementwise multiply by exp(e_t).
The constant per-step decay e^-C keeps the bf16 state centered; the exact
correction is applied in log space at the end.

v2 engine assignment (per step, per chain):
 - exp(e) is computed with a DVE 4x-rate "bitcast exp": int16 bits =
   round(184.665*e + 16248.7) are exactly the bf16 bit pattern of ~exp(e)
   (mean-corrected Schraudolph; |rel err| <= ~4%, fine for a loss summed
   over batch).  This frees the Scalar engine entirely.
 - The PSUM->SBUF hop: a tunable fraction of chain-steps goes
   PSUM -(ScalarE Copy)-> SBUF bf16 -(DVE 2x bf16 tensor_tensor)-> state,
   the rest take the direct 1x DVE tensor_tensor(PSUM, ep) path.  This
   splits the per-step elementwise work across both PSUM-capable engines.

Sharding: batch 1024 -> 4 shards x 256; time 512 -> forward half (t=0..255)
and backward half (t=511..256, reversed) = 8 cores, meeting in the middle:
  log Z_b = log( F[:,b].T @ exp(trans) @ R[:,b] ) + 2*(S-1)*C
where F = fwd exp-state after t=255, R = bwd exp-state after t=256. The tiny
[64x64x256] bridge per shard is done on host in f64 (stability), along with
the O(B) final add/sum — everything O(L*B*T) runs on device.

Numerator: gold-path gathers (pure indexing) are marshaled on host
(np.take_along_axis / fancy indexing); their O(L*B) reduction runs on device.

Host-side work is indexing/layout/dtype marshaling only, plus the O(B)
finalize.
"""

import os

import ml_dtypes
import numpy as np

import concourse.bass as bass
import concourse.bacc as bacc
import concourse.mybir as mybir
from concourse.bass_utils import run_bass_kernel_spmd
from concourse.tile import TileContext

BF16 = ml_dtypes.bfloat16

L, B, T = 512, 1024, 64
NCORES = 8
NSHARDS = 4                  # batch shards; cores 0-3 fwd, 4-7 bwd
BL = B // NSHARDS            # 256 batch columns per core
S = int(os.environ.get("CRF_STEPS", str(L // 2)))   # tiles per core (256)
NCH = 2                      # independent chains per core
G = 2                        # tag-groups stacked on partitions (blockdiag weights)
P = G * T                    # 128 partitions
CW = BL // (G * NCH)         # free columns per chain tile (64)
SB = int(os.environ.get("CRF_SB", "16"))            # emission steps per DMA chunk
DECAY = 4.66                 # per-matmul-step exp-space decay (keeps state centered)

# bitcast-exp constants: int16 bits of bf16(exp(e)) ~= EXP_C1*e + EXP_C2
EXP_C1 = 128.0 / float(np.log(2.0))       # 184.6650
EXP_C2 = 16256.0 - 7.33                   # mean-corrected Schraudolph bias

# routing: chain-step goes via ScalarE copy only if DIRECT_EVERY>1 and
# (t % DIRECT_EVERY) != 0; DIRECT_EVERY<=1 means always-direct DVE path.
DIRECT_EVERY = int(os.environ.get("CRF_DIRECT_EVERY", "1"))
WARMUP = int(os.environ.get("CRF_WARMUP", "24"))
EXP_MODE = os.environ.get("CRF_EXP_MODE", "act")    # act | dve
SWAP_TT = bool(int(os.environ.get("CRF_SWAP_TT", "0")))  # in0/in1 order of hop TT

_COMPILED = {}
LAST_RUN = {}


def _build_nc():
    nc = bacc.Bacc("TRN2", target_bir_lowering=False, debug=False)
    f32 = mybir.dt.float32
    bf16 = mybir.dt.bfloat16
    i16 = mybir.dt.int16

    assert S % SB == 0 or S < SB
    nch_chunks = max(1, S // SB)
    sbw = min(SB, S)
    W_ = BL // G                 # 128 free columns per step (both chains)
    ecw = sbw * W_
    emi = nc.dram_tensor("emi", [nch_chunks, P, ecw], bf16, kind="ExternalInput")
    wmat = nc.dram_tensor("wmat", [P, P], bf16, kind="ExternalInput")
    # per-partition init bias: col0 = raw start (ACT exp), col1 = EXP_C1*start+EXP_C2 (DVE)
    biasv = nc.dram_tensor("biasv", [P, 2], f32, kind="ExternalInput")
    nums = nc.dram_tensor("nums", [2, 128, 2 * S], f32, kind="ExternalInput")

    fstate = nc.dram_tensor("fstate", [P, BL // G], bf16, kind="ExternalOutput")
    numpart = nc.dram_tensor("numpart", [2, 128, 1], f32, kind="ExternalOutput")

    with TileContext(nc) as tc:
        with (
            tc.tile_pool(name="consts", bufs=1) as consts,
            tc.tile_pool(name="emi", bufs=int(os.environ.get("CRF_EMI_BUFS", "2"))) as emi_pool,
            tc.tile_pool(name="ep", bufs=int(os.environ.get("CRF_EMI_BUFS", "2"))) as ep_pool,
            tc.tile_pool(name="state", bufs=int(os.environ.get("CRF_STATE_BUFS", "2"))) as p_pool,
            tc.tile_pool(name="sp", bufs=int(os.environ.get("CRF_SP_BUFS", "2"))) as sp_pool,
            tc.tile_pool(name="psum", bufs=int(os.environ.get("CRF_PSUM_BUFS", "2")), space="PSUM") as psum_pool,
            tc.tile_pool(name="warm", bufs=1, space="PSUM") as warm_pool,
            tc.tile_pool(name="numr", bufs=1) as num_pool,
        ):
            w_tile = consts.tile([P, P], bf16)
            nc.gpsimd.dma_start(out=w_tile[:], in_=wmat[:, :])
            bias_tile = consts.tile([P, 2], f32)
            nc.gpsimd.dma_start(out=bias_tile[:], in_=biasv[:, :])

            # main exp-space scan
            p_prev = [None] * NCH
            echunk, ep_tile = None, None
            for s in range(S):
                if s % SB == 0:
                    echunk = emi_pool.tile([P, ecw], bf16, tag="et")
                    if s == 0:
                        # split chunk-0's DMA so the first steps aren't gated
                        # on the whole 512KB transfer
                        cut = 4 * W_
                        nc.sync.dma_start(
                            out=echunk[:, :cut], in_=emi[0][:, :cut]
                        )
                        nc.sync.dma_start(
                            out=echunk[:, cut:], in_=emi[0][:, cut:]
                        )
                    else:
                        nc.sync.dma_start(out=echunk[:], in_=emi[s // SB])
                    if s == 0 and WARMUP:
                        # PE warmup gated on chunk0's head: dense matmul burst
                        # ending right as the first step issues (HAM at 8/8)
                        wm = warm_pool.tile([P, 64], f32)
                        for _ in range(WARMUP):
                            nc.tensor.matmul(
                                wm[:], w_tile[:], echunk[:, :64],
                                start=True, stop=True,
                            )
                    if EXP_MODE == "dve":
                        ep_tile = ep_pool.tile([P, ecw], i16, tag="ep")
                        if s == 0:
                            for a, b in ((0, cut), (cut, ecw)):
                                nc.vector.tensor_scalar(
                                    out=ep_tile[:, a:b],
                                    in0=echunk[:, a:b],
                                    scalar1=EXP_C1,
                                    scalar2=EXP_C2,
                                    op0=mybir.AluOpType.mult,
                                    op1=mybir.AluOpType.add,
                                )
                        else:
                            nc.vector.tensor_scalar(
                                out=ep_tile[:],
                                in0=echunk[:],
                                scalar1=EXP_C1,
                                scalar2=EXP_C2,
                                op0=mybir.AluOpType.mult,
                                op1=mybir.AluOpType.add,
                            )
                    else:
                        ep_tile = ep_pool.tile([P, ecw], bf16, tag="ep")
                        if s == 0:
                            # head-slice exp only; the rest is emitted after
                            # the p0 inits so they aren't queued behind it
                            nc.scalar.activation(
                                ep_tile[:, :cut], echunk[:, :cut],
                                mybir.ActivationFunctionType.Exp,
                            )
                        else:
                            nc.scalar.activation(
                                ep_tile[:], echunk[:],
                                mybir.ActivationFunctionType.Exp,
                            )
                base = (s % SB) * W_

                def ep_slice(cn):
                    sl = ep_tile[:, base + cn * CW : base + (cn + 1) * CW]
                    return sl.bitcast(bf16) if EXP_MODE == "dve" else sl

                if s == 0:
                    for cn in range(NCH):
                        if EXP_MODE == "dve":
                            # p0 bits = EXP_C1*e0 + (EXP_C1*start + EXP_C2)
                            p0 = p_pool.tile([P, CW], i16, tag=f"p{cn}")
                            nc.vector.tensor_scalar(
                                out=p0[:],
                                in0=echunk[:, cn * CW : (cn + 1) * CW],
                                scalar1=EXP_C1,
                                scalar2=bias_tile[:, 1:2],
                                op0=mybir.AluOpType.mult,
                                op1=mybir.AluOpType.add,
                            )
                            p_prev[cn] = p0[:].bitcast(bf16)
                        else:
                            p0 = p_pool.tile([P, CW], bf16, tag=f"p{cn}")
                            nc.scalar.activation(
                                p0[:],
                                echunk[:, cn * CW : (cn + 1) * CW],
                                mybir.ActivationFunctionType.Exp,
                                bias=bias_tile[:, 0:1],
                            )
                            p_prev[cn] = p0[:]
                    if EXP_MODE != "dve":
                        nc.scalar.activation(
                            ep_tile[:, cut:], echunk[:, cut:],
                            mybir.ActivationFunctionType.Exp,
                        )
                    continue
                for cn in range(NCH):
                    m = psum_pool.tile([P, CW], f32, tag=f"m{cn}")
                    nc.tensor.matmul(
                        m[:], w_tile[:], p_prev[cn], start=True, stop=True
                    )
                    pn = p_pool.tile([P, CW], bf16, tag=f"p{cn}")
                    t_idx = s * NCH + cn
                    if DIRECT_EVERY <= 1 or (t_idx % DIRECT_EVERY == 0):
                        # direct: 1x DVE tensor_tensor from PSUM
                        if SWAP_TT:
                            nc.vector.tensor_tensor(
                                out=pn[:], in0=ep_slice(cn), in1=m[:],
                                op=mybir.AluOpType.mult,
                            )
                        else:
                            nc.vector.tensor_tensor(
                                out=pn[:], in0=m[:], in1=ep_slice(cn),
                                op=mybir.AluOpType.mult,
                            )
                    else:
                        # ScalarE hop + 2x bf16 DVE multiply
                        sp = sp_pool.tile([P, CW], bf16, tag=f"s{cn}")
                        nc.scalar.activation(
                            sp[:], m[:], mybir.ActivationFunctionType.Copy
                        )
                        nc.vector.tensor_tensor(
                            out=pn[:], in0=sp[:], in1=ep_slice(cn),
                            op=mybir.AluOpType.mult,
                        )
                    p_prev[cn] = pn[:]

            for cn in range(NCH):
                # split across queues so the two final DMAs run in parallel
                dma_q = nc.sync if cn == 0 else nc.gpsimd
                dma_q.dma_start(
                    out=fstate[:, cn * CW : (cn + 1) * CW], in_=p_prev[cn]
                )

            # numerator reduction after the scan (keeps its DMA off the
            # critical prologue path and its reduce off the busy DVE: the
            # ScalarE accumulator does the row sum during the scan tail)
            for h in range(2):
                ntile = num_pool.tile([128, 2 * S], f32, tag="ntile")
                nc.gpsimd.dma_start(out=ntile[:], in_=nums[h])
                nred = num_pool.tile([128, 1], f32, tag="nred")
                nc.scalar.activation(
                    ntile[:], ntile[:], mybir.ActivationFunctionType.Copy,
                    accum_out=nred[:],
                )
                nc.gpsimd.dma_start(out=numpart[h], in_=nred[:])
    nc.compile()
    return nc


def kernel(emissions, tags, mask, start_transitions, end_transitions, transitions):
    emissions = np.asarray(emissions, dtype=np.float32)          # (L, B, T)
    tags = np.asarray(tags).astype(np.int64)                     # (L, B)
    mask = np.asarray(mask)
    start_transitions = np.asarray(start_transitions, dtype=np.float32)
    end_transitions = np.asarray(end_transitions, dtype=np.float32)
    transitions = np.asarray(transitions, dtype=np.float32)
    assert bool(mask.all()), "kernel specialized for all-ones mask"

    half = L // 2

    # ---- host marshaling: layout + dtype only ----
    # gold-path gathers (indexing only; reductions happen on device)
    EG = np.take_along_axis(emissions, tags[:, :, None], axis=2)[:, :, 0]  # (L,B)
    TRS = np.zeros((L, B), np.float32)
    TRS[1:] = transitions[tags[:-1], tags[1:]]
    SG = start_transitions[tags[0]]
    ENG = end_transitions[tags[-1]]

    def blockdiag(w):
        wb = np.zeros((P, P), np.float32)
        wb[:T, :T] = w
        wb[T:, T:] = w
        return wb.astype(BF16)

    Wf = blockdiag(np.exp(transitions - DECAY))       # fwd lhsT [cur, next] x2
    Wb = blockdiag(np.exp(transitions.T - DECAY))     # bwd lhsT [next, cur] x2
    # per-partition p0-bias: col0 raw (ACT exp bias), col1 scaled (DVE bitcast-exp)
    def mk_bias(v):
        vv = np.concatenate([v, v])
        return np.stack([vv, EXP_C1 * vv + EXP_C2], axis=1).astype(np.float32)

    bias_f = mk_bias(start_transitions)
    bias_b = mk_bias(end_transitions)

    def stack_emi(slab):
        # slab (S, 256, 64) f32, b_local = 128c + 64g + j -> [chunk, 64g+k, (s%SB, 64c+j)]
        r = slab.reshape(S, 2, G, T, T)               # (S, c, g, j, k)
        r = r.transpose(0, 2, 4, 1, 3)                # (S, g, k, c, j)
        r = r.reshape(S, P, BL // G)
        sb = min(SB, S)
        r = r.reshape(S // sb, sb, P, BL // G).transpose(0, 2, 1, 3)
        return np.ascontiguousarray(
            r.reshape(S // sb, P, sb * (BL // G))
        ).astype(BF16)

    in_maps = []
    for core in range(NCORES):
        sh = core % NSHARDS
        is_bwd = core >= NSHARDS
        bsl = slice(sh * BL, (sh + 1) * BL)
        if not is_bwd:
            emi_c = stack_emi(emissions[:half, bsl][:S])
            numc = (EG[:half, bsl], TRS[:half, bsl])
        else:
            emi_c = stack_emi(emissions[half:, bsl][::-1][:S])
            numc = (EG[half:, bsl], TRS[half:, bsl])
        # nums layout: [half-of-shard h, 128 rows, EG(S) || TRS(S)]
        nums_c = np.empty((2, 128, 2 * S), np.float32)
        for h in range(2):
            rows = slice(h * 128, (h + 1) * 128)
            nums_c[h, :, :S] = numc[0][:S, rows].T
            nums_c[h, :, S:] = numc[1][:S, rows].T
        in_maps.append(
            {
                "emi": emi_c,
                "wmat": Wb if is_bwd else Wf,
                "biasv": bias_b if is_bwd else bias_f,
                "nums": nums_c,
            }
        )

    if "nc" not in _COMPILED:
        _COMPILED["nc"] = _build_nc()
    res = run_bass_kernel_spmd(
        _COMPILED["nc"],
        in_maps,
        list(range(NCORES)),
        trace=bool(int(os.environ.get("CRF_TRACE", "0"))),
    )
    LAST_RUN["exec_time_ns"] = res.exec_time_ns
    LAST_RUN["profile_json"] = res.profile_json
    outs = res.results

    # ---- host finalize: tiny f64 bridge + O(B) sums ----
    def unstack(fs):
        # [64g+k, 64c+j] -> [k, 128c+64g+j]
        r = fs.reshape(G, T, 2, T).transpose(1, 2, 0, 3)
        return np.ascontiguousarray(r.reshape(T, BL))

    Texp = np.exp(transitions.astype(np.float64))
    total = 0.0
    for sh in range(NSHARDS):
        F = unstack(outs[sh]["fstate"]).astype(np.float64)            # (T, BL)
        R = unstack(outs[NSHARDS + sh]["fstate"]).astype(np.float64)  # (T, BL)
        z = np.einsum("ib,ij,jb->b", F, Texp, R)
        log_z = np.log(z) + 2 * (S - 1) * DECAY
        bsl = slice(sh * BL, (sh + 1) * BL)
        num = (
            outs[sh]["numpart"].reshape(BL)
            + outs[NSHARDS + sh]["numpart"].reshape(BL)
            + SG[bsl]
            + ENG[bsl]
        )
        total += float((num.astype(np.float64) - log_z).sum())
    return np.float32(total)
